# revision 13
# baseline (speedup 1.0000x reference)
"""NT-Xent / SimCLR contrastive loss on 8 Trainium2 NeuronCores.

v4: all-gather collective — each core loads ONLY its own 1024 rows.

Strategy:
  - Host: reps = concat(z_i, z_j) [8192, 512] fp32 in GLOBAL order (no
    roll). Core i receives only its rows 1024i..1024i+1023 ("myrows").
  - Device (per core):
      normalize own rows to fp8 (scale S=16) exactly as v2; store the
      u16-packed (feature-pair) block [1024, 256] u16 to DRAM in the
      scratch-permuted layout (row q = p*8 + t <-> local row t*128 + p);
      ONE AllGather over cores 0..7 -> [8192, 256] u16 (global blocks);
      xbar-transpose all 8 gathered blocks (plus own block for the
      planar lhsT deinterleave, read from the local cc_in — static).
  - Phase B: identical fp8 DoubleRow matmuls + ACT Exp(scale=2/S^2) with
    fused row-sum. The positives diagonal position depends on rank
    (global col block (rank+4)%8), so the device extracts the diagonal
    of EVERY 1024-col block into "diag" [128, 8m+g] and the HOST (which
    knows the rank) picks block (i+4)%8 and sums.
  - Self-similarity: denom = rowsum - e^2 (constant; sim_ii == 1).
  - Host: loss_i = lnsum_i - LOGIT_SCALE * sum(diag_i[:, :, (i+4)%8]);
    loss = sum(loss_i) / 8192.
"""

import sys
import threading
from unittest import mock

sys.path.insert(0, "/opt/trn_rl_repo")

import numpy as np  # noqa: E402

import concourse.tile as tile  # noqa: E402
from concourse import bacc, mybir  # noqa: E402
from concourse.bass_utils import run_bass_kernel_spmd  # noqa: E402
from concourse.hw_specs import get_activation_tables  # noqa: E402
from concourse.masks import make_identity  # noqa: E402
from contextlib import ExitStack  # noqa: E402

P = 128
D = 512
TWO_N = 8192
N_CORES = 8
ROWS_PER_CORE = TWO_N // N_CORES  # 1024
T_INV = 2.0  # 1 / temperature (0.5)
S = 16.0  # fp8 range scale
LOGIT_SCALE = T_INV / (S * S)
E_SELF = float(np.exp(T_INV))

NG = 8  # column groups (= gathered blocks)
CB = TWO_N // NG  # 1024 columns per group
TPG = ROWS_PER_CORE // P  # 8 [128, 512] row tiles in own block
MB = ROWS_PER_CORE // P  # 8 m-blocks
NKP = 2  # feature-pair chunks (256 features each)
NPB = 4  # psum blocks of 2048 columns

FP32 = mybir.dt.float32
BF16 = mybir.dt.bfloat16
FP8 = mybir.dt.float8e4
U16 = mybir.dt.uint16
AF = mybir.ActivationFunctionType
ALU = mybir.AluOpType
AX = mybir.AxisListType
DR = mybir.MatmulPerfMode.DoubleRow
RG = [[i for i in range(N_CORES)]]


def _filtered_activation_tables(arch):
    tables = get_activation_tables(arch)
    target = None
    for name, funcs in tables.items():
        if AF.Exp in funcs and AF.Ln in funcs:
            target = name
            break
    if target is None:
        return tables
    steer = {AF.Exp, AF.Ln, AF.Copy, AF.Identity}
    return {
        name: (funcs if name == target else funcs - steer)
        for name, funcs in tables.items()
    }


def _build_kernel():
    nc = bacc.Bacc("TRN2", target_bir_lowering=False, debug=False,
                   num_devices=N_CORES)
    myrows = nc.dram_tensor("myrows", [ROWS_PER_CORE, D], FP32,
                            kind="ExternalInput").ap()
    out = nc.dram_tensor("out", [1, 1], FP32, kind="ExternalOutput").ap()
    diag_out = nc.dram_tensor("diag", [P, MB * NG], FP32,
                              kind="ExternalOutput").ap()
    cc_in = nc.dram_tensor("cc_in", [ROWS_PER_CORE, NKP * P], U16).ap()
    cc_out = nc.dram_tensor("cc_out", [TWO_N, NKP * P], U16).ap()

    with tile.TileContext(nc) as tc, ExitStack() as ctx:
        rows_pool = ctx.enter_context(tc.tile_pool(name="rows", bufs=1))
        normed_pool = ctx.enter_context(tc.tile_pool(name="normed", bufs=1))
        sq_pool = ctx.enter_context(tc.tile_pool(name="sq", bufs=2))
        stats_pool = ctx.enter_context(tc.tile_pool(name="stats", bufs=1))
        repsT_pool = ctx.enter_context(tc.tile_pool(name="repsT", bufs=1))
        psum_pool = ctx.enter_context(
            tc.tile_pool(name="psum", bufs=2, space="PSUM"))
        exp_pool = ctx.enter_context(tc.tile_pool(name="exp", bufs=2))
        junk_pool = ctx.enter_context(tc.tile_pool(name="junk", bufs=2))
        epi_pool = ctx.enter_context(tc.tile_pool(name="epi", bufs=1))

        # --- constants -----------------------------------------------------
        ident = stats_pool.tile([P, P], FP32, tag="ident", name="ident")
        make_identity(nc, ident[:])
        ones = stats_pool.tile([P, 1], FP32, tag="ones", name="ones")
        nc.gpsimd.memset(ones[:], 1.0)

        rs_all = stats_pool.tile([P, MB * NPB], FP32, tag="rs", name="rs_all")
        diag_sb = stats_pool.tile([P, MB * NG], FP32, tag="diag",
                                  name="diag_sb")

        # repsT[kp][g]: [128, 1024] u16 — feature-pair chunk kp of gathered
        # block g (global rows 1024g..), scratch-permuted col q <->
        # in-block row (q%8)*128 + q//8.
        repsT = [[repsT_pool.tile([P, CB], U16, tag=f"rT{kp}_{g}",
                                  name=f"repsT_{kp}_{g}")
                  for g in range(NG)]
                 for kp in range(NKP)]
        own_repsT = [repsT_pool.tile([P, CB], U16, tag=f"rTown_{kp}",
                                     name=f"own_repsT_{kp}")
                     for kp in range(NKP)]
        repsT0 = [repsT_pool.tile([P, 2 * ROWS_PER_CORE], FP8, tag=f"rTz_{kp}",
                                  name=f"repsT0_{kp}")
                  for kp in range(NKP)]

        # --- phase A: normalize OWN rows, gather, transpose ----------------
        rows = rows_pool.tile([P, TPG * D], FP32, tag="rows", name="rows")
        src = myrows[:, :].rearrange("(t p) d -> p t d", p=P)
        for t in range(TPG):
            nc.sync.dma_start(out=rows[:, t * D:(t + 1) * D], in_=src[:, t, :])

        n2 = stats_pool.tile([P, TPG], FP32, tag="n2", name="n2")
        for t in range(TPG):
            sq = sq_pool.tile([P, D], BF16, tag="sq", name=f"sq_{t}")
            rt = rows[:, t * D:(t + 1) * D]
            nc.vector.scalar_tensor_tensor(
                out=sq[:], in0=rt, scalar=1.0, in1=rt,
                op0=ALU.mult, op1=ALU.mult, accum_out=n2[:, t:t + 1])
        lnn = stats_pool.tile([P, TPG], FP32, tag="lnn", name="lnn")
        nc.scalar.activation(lnn[:], n2[:], AF.Ln, scale=1.0 / (S * S))
        inv = stats_pool.tile([P, TPG], FP32, tag="inv", name="inv")
        nc.scalar.activation(inv[:], lnn[:], AF.Exp, scale=-0.5)

        normed = normed_pool.tile([P, TPG * D], FP8, tag="normed",
                                  name="normed")
        for t in range(TPG):
            nc.vector.tensor_scalar(
                out=normed[:, t * D:(t + 1) * D],
                in0=rows[:, t * D:(t + 1) * D],
                scalar1=inv[:, t:t + 1], scalar2=None, op0=ALU.mult)

        # store u16-packed permuted block: cc_in row q = p*8 + t
        nview = normed[:].bitcast(U16).rearrange("p (t e) -> p t e", e=D // 2)
        nc.sync.dma_start(
            out=cc_in[:, :].rearrange("(p t) c -> p t c", p=P), in_=nview)

        # own transposes (static, from local cc_in) for the lhsT path
        for kp in range(NKP):
            nc.sync.dma_start_transpose(
                own_repsT[kp][:], cc_in[:, kp * P:(kp + 1) * P])
            # deinterleave: dst[c, i*1024 + m*128 + j] = byte 2*(8j+m) + i
            nc.vector.tensor_copy(
                repsT0[kp][:].rearrange("p (two m j) -> p two m j",
                                        two=2, m=MB),
                own_repsT[kp][:].bitcast(FP8).rearrange(
                    "p (j m two) -> p two m j", two=2, m=TPG))

        # all-gather the packed blocks (HBM -> HBM)
        nc.gpsimd.collective_compute(
            kind="AllGather", op=ALU.bypass, replica_groups=RG,
            ins=[cc_in[:, :]], outs=[cc_out[:, :]])

        for g in range(NG):
            for kp in range(NKP):
                nc.sync.dma_start_transpose(
                    repsT[kp][g][:],
                    cc_out[g * CB:(g + 1) * CB, kp * P:(kp + 1) * P])

        # --- phase B: DoubleRow similarity matmuls + softmax stats ---------
        def colsel(ap_2d, m):
            # columns of m-block m sit at permuted positions q = 8*j + m
            return ap_2d.rearrange("p (j s) -> p s j", s=TPG)[:, m, :]

        for pb in range(NPB):
            for m in range(MB):
                ps = psum_pool.tile([P, 2 * CB], FP32, tag="ps",
                                    name=f"ps_{pb}_{m}")
                for kp in range(NKP):
                    lhsT = repsT0[kp][:].rearrange(
                        "p (two mj) -> p two mj", two=2)[
                        :, :, m * P:(m + 1) * P]
                    for half in range(2):
                        rhs_g = repsT[kp][2 * pb + half][:].bitcast(
                            FP8).rearrange("p (n two) -> p two n", two=2)
                        for ns in range(2):
                            nc.tensor.matmul(
                                ps[:, half * CB + ns * 512:
                                   half * CB + (ns + 1) * 512],
                                lhsT=lhsT,
                                rhs=rhs_g[:, :, ns * 512:(ns + 1) * 512],
                                start=(kp == 0), stop=(kp == NKP - 1),
                                perf_mode=DR, skip_group_check=True)
                et = exp_pool.tile([P, 2 * CB], BF16, tag="et",
                                   name=f"et_{pb}_{m}")
                nc.scalar.activation(
                    et[:], ps[:], AF.Exp, scale=LOGIT_SCALE,
                    accum_out=rs_all[:, m * NPB + pb:m * NPB + pb + 1])
                # extract the [own rows x block g] diagonal for BOTH halves;
                # host picks the positive block g* = (rank+4)%8.
                for half in range(2):
                    g = 2 * pb + half
                    junk = junk_pool.tile([P, P], FP32, tag="junk",
                                          name=f"junk_{pb}_{m}_{half}")
                    nc.vector.scalar_tensor_tensor(
                        out=junk[:], in0=colsel(
                            ps[:, half * CB:(half + 1) * CB], m),
                        scalar=1.0, in1=ident[:],
                        op0=ALU.mult, op1=ALU.mult,
                        accum_out=diag_sb[:, m * NG + g:m * NG + g + 1])

        # --- epilogue ------------------------------------------------------
        sums = epi_pool.tile([P, MB], FP32, tag="sums", name="sums")
        nc.vector.tensor_reduce(
            sums[:], rs_all[:].rearrange("p (m b) -> p m b", b=NPB),
            axis=AX.X, op=ALU.add)
        denom = epi_pool.tile([P, MB], FP32, tag="denom", name="denom")
        nc.vector.tensor_scalar_add(denom[:], sums[:], -E_SELF)
        ld = epi_pool.tile([P, MB], FP32, tag="ld", name="ld")
        nc.scalar.activation(ld[:], denom[:], AF.Ln)
        rowtot = epi_pool.tile([P, 1], FP32, tag="rowtot", name="rowtot")
        nc.vector.tensor_reduce(rowtot[:], ld[:], axis=AX.X, op=ALU.add)
        pfin = psum_pool.tile([P, 2 * CB], FP32, tag="ps", name="pfin")
        nc.tensor.matmul(pfin[:1, :1], lhsT=ones[:], rhs=rowtot[:])
        out_sb = epi_pool.tile([1, 1], FP32, tag="osb", name="out_sb")
        nc.vector.tensor_copy(out_sb[:], pfin[:1, :1])
        nc.sync.dma_start(out=out[:, :], in_=out_sb[:])
        nc.sync.dma_start(out=diag_out[:, :], in_=diag_sb[:])

    with mock.patch("concourse.bacc.get_activation_tables",
                    _filtered_activation_tables):
        nc.compile()
    return nc


_CACHE_LOCK = threading.Lock()
_CACHED_NC = None


def _get_nc():
    global _CACHED_NC
    with _CACHE_LOCK:
        if _CACHED_NC is None:
            _CACHED_NC = _build_kernel()
        return _CACHED_NC


def _run(inputs, trace=False):
    z_i = np.asarray(inputs["z_i"], dtype=np.float32)
    z_j = np.asarray(inputs["z_j"], dtype=np.float32)
    reps = np.concatenate([z_i, z_j], axis=0)
    in_maps = [
        {"myrows": np.ascontiguousarray(
            reps[ROWS_PER_CORE * i:ROWS_PER_CORE * (i + 1)])}
        for i in range(N_CORES)
    ]
    nc = _get_nc()
    res = run_bass_kernel_spmd(nc, in_maps, list(range(N_CORES)), trace=trace)
    total = 0.0
    for i in range(N_CORES):
        lnsum = float(res.results[i]["out"][0, 0])
        diag = np.asarray(res.results[i]["diag"], dtype=np.float64)
        g_star = (i + 4) % NG
        pos_sum = diag.reshape(P, MB, NG)[:, :, g_star].sum()
        total += lnsum - LOGIT_SCALE * pos_sum
    loss = np.float32(total / TWO_N)
    return loss, res


def kernel(**inputs):
    loss, _ = _run(inputs, trace=False)
    return np.asarray(loss, dtype=np.float32)


# revision 14
# speedup vs baseline: 1.0421x; 1.0421x over previous
"""NT-Xent / SimCLR contrastive loss on 8 Trainium2 NeuronCores.

v4: all-gather collective — each core loads ONLY its own 1024 rows.

Strategy:
  - Host: reps = concat(z_i, z_j) [8192, 512] fp32 in GLOBAL order (no
    roll). Core i receives only its rows 1024i..1024i+1023 ("myrows").
  - Device (per core):
      normalize own rows to fp8 (scale S=16) exactly as v2; store the
      u16-packed (feature-pair) block [1024, 256] u16 to DRAM in the
      scratch-permuted layout (row q = p*8 + t <-> local row t*128 + p);
      ONE AllGather over cores 0..7 -> [8192, 256] u16 (global blocks);
      xbar-transpose all 8 gathered blocks (plus own block for the
      planar lhsT deinterleave, read from the local cc_in — static).
  - Phase B: identical fp8 DoubleRow matmuls + ACT Exp(scale=2/S^2) with
    fused row-sum. The positives diagonal position depends on rank
    (global col block (rank+4)%8), so the device extracts the diagonal
    of EVERY 1024-col block into "diag" [128, 8m+g] and the HOST (which
    knows the rank) picks block (i+4)%8 and sums.
  - Self-similarity: denom = rowsum - e^2 (constant; sim_ii == 1).
  - Host: loss_i = lnsum_i - LOGIT_SCALE * sum(diag_i[:, :, (i+4)%8]);
    loss = sum(loss_i) / 8192.
"""

import sys
import threading
from unittest import mock

sys.path.insert(0, "/opt/trn_rl_repo")

import numpy as np  # noqa: E402

import concourse.tile as tile  # noqa: E402
from concourse import bacc, mybir  # noqa: E402
from concourse.bass_utils import run_bass_kernel_spmd  # noqa: E402
from concourse.hw_specs import get_activation_tables  # noqa: E402
from concourse.masks import make_identity  # noqa: E402
from contextlib import ExitStack  # noqa: E402

P = 128
D = 512
TWO_N = 8192
N_CORES = 8
ROWS_PER_CORE = TWO_N // N_CORES  # 1024
T_INV = 2.0  # 1 / temperature (0.5)
S = 16.0  # fp8 range scale
LOGIT_SCALE = T_INV / (S * S)
E_SELF = float(np.exp(T_INV))

NG = 8  # column groups (= gathered blocks)
CB = TWO_N // NG  # 1024 columns per group
TPG = ROWS_PER_CORE // P  # 8 [128, 512] row tiles in own block
MB = ROWS_PER_CORE // P  # 8 m-blocks
NKP = 2  # feature-pair chunks (256 features each)
NPB = 4  # psum blocks of 2048 columns

FP32 = mybir.dt.float32
BF16 = mybir.dt.bfloat16
FP8 = mybir.dt.float8e4
U16 = mybir.dt.uint16
AF = mybir.ActivationFunctionType
ALU = mybir.AluOpType
AX = mybir.AxisListType
DR = mybir.MatmulPerfMode.DoubleRow
RG = [[i for i in range(N_CORES)]]


def _filtered_activation_tables(arch):
    tables = get_activation_tables(arch)
    target = None
    for name, funcs in tables.items():
        if AF.Exp in funcs and AF.Ln in funcs:
            target = name
            break
    if target is None:
        return tables
    steer = {AF.Exp, AF.Ln, AF.Copy, AF.Identity}
    return {
        name: (funcs if name == target else funcs - steer)
        for name, funcs in tables.items()
    }


def _build_kernel():
    nc = bacc.Bacc("TRN2", target_bir_lowering=False, debug=False,
                   num_devices=N_CORES)
    myrows = nc.dram_tensor("myrows", [ROWS_PER_CORE, D], FP32,
                            kind="ExternalInput").ap()
    out = nc.dram_tensor("out", [1, 1], FP32, kind="ExternalOutput").ap()
    diag_out = nc.dram_tensor("diag", [P, MB * NG], FP32,
                              kind="ExternalOutput").ap()
    cc_in = nc.dram_tensor("cc_in", [ROWS_PER_CORE, NKP * P], U16).ap()
    cc_out = nc.dram_tensor("cc_out", [TWO_N, NKP * P], U16,
                            addr_space="Shared").ap()

    with tile.TileContext(nc) as tc, ExitStack() as ctx:
        rows_pool = ctx.enter_context(tc.tile_pool(name="rows", bufs=1))
        normed_pool = ctx.enter_context(tc.tile_pool(name="normed", bufs=1))
        sq_pool = ctx.enter_context(tc.tile_pool(name="sq", bufs=2))
        stats_pool = ctx.enter_context(tc.tile_pool(name="stats", bufs=1))
        repsT_pool = ctx.enter_context(tc.tile_pool(name="repsT", bufs=1))
        psum_pool = ctx.enter_context(
            tc.tile_pool(name="psum", bufs=2, space="PSUM"))
        exp_pool = ctx.enter_context(tc.tile_pool(name="exp", bufs=2))
        junk_pool = ctx.enter_context(tc.tile_pool(name="junk", bufs=2))
        epi_pool = ctx.enter_context(tc.tile_pool(name="epi", bufs=1))

        # --- constants -----------------------------------------------------
        ident = stats_pool.tile([P, P], FP32, tag="ident", name="ident")
        make_identity(nc, ident[:])
        ones = stats_pool.tile([P, 1], FP32, tag="ones", name="ones")
        nc.gpsimd.memset(ones[:], 1.0)

        rs_all = stats_pool.tile([P, MB * NPB], FP32, tag="rs", name="rs_all")
        diag_sb = stats_pool.tile([P, MB * NG], FP32, tag="diag",
                                  name="diag_sb")

        # repsT[kp][g]: [128, 1024] u16 — feature-pair chunk kp of gathered
        # block g (global rows 1024g..), scratch-permuted col q <->
        # in-block row (q%8)*128 + q//8.
        repsT = [[repsT_pool.tile([P, CB], U16, tag=f"rT{kp}_{g}",
                                  name=f"repsT_{kp}_{g}")
                  for g in range(NG)]
                 for kp in range(NKP)]
        own_repsT = [repsT_pool.tile([P, CB], U16, tag=f"rTown_{kp}",
                                     name=f"own_repsT_{kp}")
                     for kp in range(NKP)]
        repsT0 = [repsT_pool.tile([P, 2 * ROWS_PER_CORE], FP8, tag=f"rTz_{kp}",
                                  name=f"repsT0_{kp}")
                  for kp in range(NKP)]

        # --- phase A: normalize OWN rows, gather, transpose ----------------
        rows = rows_pool.tile([P, TPG * D], FP32, tag="rows", name="rows")
        src = myrows[:, :].rearrange("(t p) d -> p t d", p=P)
        for t in range(TPG):
            nc.sync.dma_start(out=rows[:, t * D:(t + 1) * D], in_=src[:, t, :])

        n2 = stats_pool.tile([P, TPG], FP32, tag="n2", name="n2")
        for t in range(TPG):
            sq = sq_pool.tile([P, D], BF16, tag="sq", name=f"sq_{t}")
            rt = rows[:, t * D:(t + 1) * D]
            nc.vector.scalar_tensor_tensor(
                out=sq[:], in0=rt, scalar=1.0, in1=rt,
                op0=ALU.mult, op1=ALU.mult, accum_out=n2[:, t:t + 1])
        lnn = stats_pool.tile([P, TPG], FP32, tag="lnn", name="lnn")
        nc.scalar.activation(lnn[:], n2[:], AF.Ln, scale=1.0 / (S * S))
        inv = stats_pool.tile([P, TPG], FP32, tag="inv", name="inv")
        nc.scalar.activation(inv[:], lnn[:], AF.Exp, scale=-0.5)

        normed = normed_pool.tile([P, TPG * D], FP8, tag="normed",
                                  name="normed")
        for t in range(TPG):
            nc.vector.tensor_scalar(
                out=normed[:, t * D:(t + 1) * D],
                in0=rows[:, t * D:(t + 1) * D],
                scalar1=inv[:, t:t + 1], scalar2=None, op0=ALU.mult)

        # store u16-packed permuted block: cc_in row q = p*8 + t
        nview = normed[:].bitcast(U16).rearrange("p (t e) -> p t e", e=D // 2)
        nc.sync.dma_start(
            out=cc_in[:, :].rearrange("(p t) c -> p t c", p=P), in_=nview)

        # own transposes (static, from local cc_in) for the lhsT path
        for kp in range(NKP):
            nc.sync.dma_start_transpose(
                own_repsT[kp][:], cc_in[:, kp * P:(kp + 1) * P])
            # deinterleave: dst[c, i*1024 + m*128 + j] = byte 2*(8j+m) + i
            nc.vector.tensor_copy(
                repsT0[kp][:].rearrange("p (two m j) -> p two m j",
                                        two=2, m=MB),
                own_repsT[kp][:].bitcast(FP8).rearrange(
                    "p (j m two) -> p two m j", two=2, m=TPG))

        # all-gather the packed blocks (HBM -> HBM)
        nc.gpsimd.collective_compute(
            kind="AllGather", op=ALU.bypass, replica_groups=RG,
            ins=[cc_in[:, :]], outs=[cc_out[:, :]])

        for g in range(NG):
            for kp in range(NKP):
                nc.sync.dma_start_transpose(
                    repsT[kp][g][:],
                    cc_out[g * CB:(g + 1) * CB, kp * P:(kp + 1) * P])

        # --- phase B: DoubleRow similarity matmuls + softmax stats ---------
        def colsel(ap_2d, m):
            # columns of m-block m sit at permuted positions q = 8*j + m
            return ap_2d.rearrange("p (j s) -> p s j", s=TPG)[:, m, :]

        for pb in range(NPB):
            for m in range(MB):
                ps = psum_pool.tile([P, 2 * CB], FP32, tag="ps",
                                    name=f"ps_{pb}_{m}")
                for kp in range(NKP):
                    lhsT = repsT0[kp][:].rearrange(
                        "p (two mj) -> p two mj", two=2)[
                        :, :, m * P:(m + 1) * P]
                    for half in range(2):
                        rhs_g = repsT[kp][2 * pb + half][:].bitcast(
                            FP8).rearrange("p (n two) -> p two n", two=2)
                        for ns in range(2):
                            nc.tensor.matmul(
                                ps[:, half * CB + ns * 512:
                                   half * CB + (ns + 1) * 512],
                                lhsT=lhsT,
                                rhs=rhs_g[:, :, ns * 512:(ns + 1) * 512],
                                start=(kp == 0), stop=(kp == NKP - 1),
                                perf_mode=DR, skip_group_check=True)
                et = exp_pool.tile([P, 2 * CB], BF16, tag="et",
                                   name=f"et_{pb}_{m}")
                nc.scalar.activation(
                    et[:], ps[:], AF.Exp, scale=LOGIT_SCALE,
                    accum_out=rs_all[:, m * NPB + pb:m * NPB + pb + 1])
                # extract the [own rows x block g] diagonal for BOTH halves;
                # host picks the positive block g* = (rank+4)%8.
                for half in range(2):
                    g = 2 * pb + half
                    junk = junk_pool.tile([P, P], FP32, tag="junk",
                                          name=f"junk_{pb}_{m}_{half}")
                    nc.vector.scalar_tensor_tensor(
                        out=junk[:], in0=colsel(
                            ps[:, half * CB:(half + 1) * CB], m),
                        scalar=1.0, in1=ident[:],
                        op0=ALU.mult, op1=ALU.mult,
                        accum_out=diag_sb[:, m * NG + g:m * NG + g + 1])

        # --- epilogue ------------------------------------------------------
        sums = epi_pool.tile([P, MB], FP32, tag="sums", name="sums")
        nc.vector.tensor_reduce(
            sums[:], rs_all[:].rearrange("p (m b) -> p m b", b=NPB),
            axis=AX.X, op=ALU.add)
        denom = epi_pool.tile([P, MB], FP32, tag="denom", name="denom")
        nc.vector.tensor_scalar_add(denom[:], sums[:], -E_SELF)
        ld = epi_pool.tile([P, MB], FP32, tag="ld", name="ld")
        nc.scalar.activation(ld[:], denom[:], AF.Ln)
        rowtot = epi_pool.tile([P, 1], FP32, tag="rowtot", name="rowtot")
        nc.vector.tensor_reduce(rowtot[:], ld[:], axis=AX.X, op=ALU.add)
        pfin = psum_pool.tile([P, 2 * CB], FP32, tag="ps", name="pfin")
        nc.tensor.matmul(pfin[:1, :1], lhsT=ones[:], rhs=rowtot[:])
        out_sb = epi_pool.tile([1, 1], FP32, tag="osb", name="out_sb")
        nc.vector.tensor_copy(out_sb[:], pfin[:1, :1])
        nc.sync.dma_start(out=out[:, :], in_=out_sb[:])
        nc.sync.dma_start(out=diag_out[:, :], in_=diag_sb[:])

    with mock.patch("concourse.bacc.get_activation_tables",
                    _filtered_activation_tables):
        nc.compile()
    return nc


_CACHE_LOCK = threading.Lock()
_CACHED_NC = None


def _get_nc():
    global _CACHED_NC
    with _CACHE_LOCK:
        if _CACHED_NC is None:
            _CACHED_NC = _build_kernel()
        return _CACHED_NC


def _run(inputs, trace=False):
    z_i = np.asarray(inputs["z_i"], dtype=np.float32)
    z_j = np.asarray(inputs["z_j"], dtype=np.float32)
    reps = np.concatenate([z_i, z_j], axis=0)
    in_maps = [
        {"myrows": np.ascontiguousarray(
            reps[ROWS_PER_CORE * i:ROWS_PER_CORE * (i + 1)])}
        for i in range(N_CORES)
    ]
    nc = _get_nc()
    res = run_bass_kernel_spmd(nc, in_maps, list(range(N_CORES)), trace=trace)
    total = 0.0
    for i in range(N_CORES):
        lnsum = float(res.results[i]["out"][0, 0])
        diag = np.asarray(res.results[i]["diag"], dtype=np.float64)
        g_star = (i + 4) % NG
        pos_sum = diag.reshape(P, MB, NG)[:, :, g_star].sum()
        total += lnsum - LOGIT_SCALE * pos_sum
    loss = np.float32(total / TWO_N)
    return loss, res


def kernel(**inputs):
    loss, _ = _run(inputs, trace=False)
    return np.asarray(loss, dtype=np.float32)
